# revision 36
# baseline (speedup 1.0000x reference)
"""AttentiveMatch kernel for Trainium2 (8 NeuronCores, data-parallel over batch).

Reference math (per batch):
    pn = l2norm(p); qn = l2norm(q)
    w  = -(pn @ qn^T) / D          # [S,S]
    mv = (w @ q) / S               # [S,D]
    mn = l2norm(mv)
    out = -mean(pn * mn, -1)       # [S]

Folded device pipeline (scalars folded, sign flips cancel):
    qs   = sqrt(1/|q_j|) * q_j                      (host)
    G'   = qs @ p^T                 [S,S]  fp8 DoubleRow matmul (G'[j,i])
    g8   = fp8(G')                  PSUM->SBUF copy
    dot_i = sum_j g8[j,i]^2         square (ACT) + adds + ones-matmul
    M    = sum_j g8[j,i] qs[j,d]    [i,d]  fp8 DoubleRow matmul
    ss_i = sum_d M[i,d]^2           square-accumulate along free dim
    out_i = (1/(D |p_i|)) dot_i / sqrt(ss_i)

Each core handles 8 batches; inputs shipped as fp8(e4m3) in transposed
(d-major) and natural (j-major) layouts; all accumulation fp32.
"""

import os
import sys

for _p in ("/opt/trn_rl_repo",):
    if _p not in sys.path:
        sys.path.append(_p)

import numpy as np
import ml_dtypes

import concourse.bacc as bacc
import concourse.mybir as mybir
import concourse.tile as tile
from concourse.bass_utils import run_bass_kernel_spmd

B, S, D = 64, 512, 768
NCORES = 8
BP = B // NCORES          # batches per core
ST = S // 128             # s tiles (4)
KT = D // 128             # d tiles (6)
DC = 2                    # d chunks for mm2 output (2 x 384)
DW = D // DC              # 384
F32 = mybir.dt.float32
BF16 = mybir.dt.bfloat16
F8 = mybir.dt.float8e4
AF = mybir.ActivationFunctionType
ALU = mybir.AluOpType
DR = mybir.MatmulPerfMode.DoubleRow

_NC = None


def _build(ncores=NCORES, do_compile=True):
    nc = bacc.Bacc("TRN2", target_bir_lowering=False, debug=False, num_devices=ncores)
    # transposed layouts: [b, part, k, s] with d = k*128 + part
    pt_d = nc.dram_tensor("pt8", [BP, 128, KT, S], F8, kind="ExternalInput")
    qt_d = nc.dram_tensor("qt8", [BP, 128, KT, S], F8, kind="ExternalInput")
    # natural layout: [b, part, js, d] with j = js*128 + part
    qh_d = nc.dram_tensor("qh8", [BP, 128, ST, D], F8, kind="ExternalInput")
    # 1/(D*|p_i|) at [part, b*ST + t], i = t*128 + part
    rpc_d = nc.dram_tensor("rpc", [128, BP * ST], F32, kind="ExternalInput")
    out_d = nc.dram_tensor("out", [128, BP * ST], F32, kind="ExternalOutput")

    with tile.TileContext(nc) as tc:
        with (
            tc.tile_pool(name="cst", bufs=1) as cst,
            tc.tile_pool(name="inp", bufs=3) as inp,
            tc.tile_pool(name="g8p", bufs=2) as g8p,
            tc.tile_pool(name="hpp", bufs=2) as hpp,
            tc.tile_pool(name="scr", bufs=2) as scr,
            tc.tile_pool(name="res", bufs=1) as res,
            tc.tile_pool(name="gps", bufs=3, space="PSUM") as gps,
            tc.tile_pool(name="mps", bufs=2, space="PSUM") as mps,
            tc.tile_pool(name="dps", bufs=1, space="PSUM") as dps,
        ):
            ones16 = cst.tile([128, 1], BF16)
            nc.gpsimd.memset(ones16[:], 1.0)
            rpc = cst.tile([128, BP * ST], F32)
            nc.gpsimd.dma_start(rpc[:], rpc_d[:])
            onef = cst.tile([128, 1], F32)
            nc.gpsimd.memset(onef[:], 1.0)
            warm = cst.tile([128, 1], F32)
            # PE warmup: dummy matmuls during the initial DMA wait fill the
            # idle window and trip the HAM clock-gate to full speed early
            wsrc = cst.tile([128, S], BF16)
            nc.vector.memset(wsrc[:], 1.0)
            wps = mps.tile([128, DC, 512], F32, tag="mc")
            NWARM = 7
            for i in range(NWARM):
                nc.tensor.matmul(
                    wps[0:1, 0, 0:S], lhsT=ones16[:], rhs=wsrc[:],
                    start=(i == 0), stop=(i == NWARM - 1),
                )

            # per-i dot products, transposed: col b*ST + t, i = t*128 + part
            dotT = dps.tile([128, 512], F32)
            # ss accumulator columns [i-part, b*ST + ib]
            ssc = res.tile([128, BP * ST], F32)

            # mm2 for batch st["b"], one i-block; interleaved with the NEXT
            # batch's mm1 so PSUM drains (ACT ss) get the whole iteration
            def mm2_step(st, ib):
                g8_, qh_ = st["g8"], st["qh"]
                mc = mps.tile([128, DC, 512], F32, tag="mc")
                for js in range(0, ST, 2):
                    for dc in range(DC):
                        nc.tensor.matmul(
                            mc[:, dc, 0:DW],
                            lhsT=g8_[:, js:js + 2, ib * 128:(ib + 1) * 128],
                            rhs=qh_[:, js:js + 2, dc * DW:(dc + 1) * DW],
                            start=(js == 0), stop=(js == ST - 2),
                            perf_mode=DR,
                        )
                col = st["b"] * ST + ib
                # ss: one ACT square + accumulate over both chunks (3D AP)
                s2 = scr.tile([128, DC * DW], BF16, tag="s2")
                nc.scalar.activation(
                    s2[:], mc[:, :, 0:DW], AF.Square,
                    accum_out=ssc[:, col:col + 1],
                )

            def dot_finish(st):
                bp_, hs = st["b"], st["hs"]
                for t in range(ST):
                    col = bp_ * ST + t
                    nc.tensor.matmul(
                        dotT[:, col:col + 1],
                        lhsT=hs[:, t * 128:(t + 1) * 128],
                        rhs=ones16[:],
                        start=(bp_ == 0 and t == 0),
                        stop=(bp_ == BP - 1 and t == ST - 1),
                        skip_group_check=True,
                    )
                # finals in two halves: out = dot * rpc / sqrt(ss)
                if bp_ in (BP // 2 - 1, BP - 1):
                    h0 = (0 if bp_ < BP // 2 else BP // 2) * ST
                    h1 = (bp_ + 1) * ST
                    sd = res.tile([128, BP * ST], F32, tag="sd")
                    nc.scalar.activation(sd[:, h0:h1], ssc[:, h0:h1], AF.Sqrt)
                    rs = res.tile([128, BP * ST], F32, tag="rs")
                    nc.vector.reciprocal(rs[:, h0:h1], sd[:, h0:h1])
                    w1 = res.tile([128, BP * ST], F32, tag="w1")
                    nc.vector.tensor_tensor(
                        w1[:, h0:h1], dotT[:, h0:h1], rs[:, h0:h1], ALU.mult
                    )
                    wd = res.tile([128, BP * ST], F32, tag="wd")
                    nc.vector.tensor_tensor(
                        wd[:, h0:h1], w1[:, h0:h1], rpc[:, h0:h1], ALU.mult
                    )
                    nc.sync.dma_start(out_d[:, h0:h1], wd[:, h0:h1])

            state = None
            for b in range(BP):
                # batch 0 lands chunked so mm1 starts on the first arrivals
                nch = 3 if b == 0 else 1
                w = KT // nch
                qt_c = []
                pt_c = []
                if b == 0:
                    ring_q, ring_p = nc.sync, nc.scalar
                else:
                    ring_q = ring_p = nc.sync
                for c in range(nch):
                    qc = inp.tile([128, w, S], F8, tag=f"qt{c}_{nch}")
                    ring_q.dma_start(qc[:], qt_d[b, :, c * w:(c + 1) * w, :])
                    pc = inp.tile([128, w, S], F8, tag=f"pt{c}_{nch}")
                    ring_p.dma_start(pc[:], pt_d[b, :, c * w:(c + 1) * w, :])
                    qt_c.append(qc)
                    pt_c.append(pc)
                qh_t = inp.tile([128, ST, D], F8, tag="qh")
                # b0's qh rides the scalar ring FIFO behind the pt chunks so
                # it does not steal HBM bandwidth from the critical chunks
                (nc.scalar if b == 0 else nc.gpsimd).dma_start(qh_t[:], qh_d[b])
                if state is not None:
                    # finish prev batch's h' pair-sums on the idle engine;
                    # inputs were ready last iteration so this hides fully
                    ha2 = scr.tile([128, S], BF16, tag="ha2")
                    nc.gpsimd.tensor_tensor(
                        ha2[:], state["hp"][2][:], state["hp"][3][:], ALU.add
                    )
                    hs = scr.tile([128, S], BF16, tag="hs")
                    nc.gpsimd.tensor_tensor(hs[:], state["ha"][:], ha2[:], ALU.add)
                    state["hs"] = hs
                if b == 0:
                    # preload ACT function tables during the initial DMA wait
                    nc.scalar.activation(warm[:], onef[:], AF.Square)
                    nc.scalar.activation(warm[:], onef[:], AF.Sqrt)

                g8 = g8p.tile([128, ST, S], F8, tag="g8")
                hp0 = hpp.tile([128, S], BF16, tag="hp0")
                hp1 = hpp.tile([128, S], BF16, tag="hp1")
                hp2 = hpp.tile([128, S], BF16, tag="hp2")
                hp3 = hpp.tile([128, S], BF16, tag="hp3")
                hp = [hp0, hp1, hp2, hp3]
                ha = scr.tile([128, S], BF16, tag="ha")

                # mm1: G'[j,i] = sum_d qs[j,d] p[i,d]  (fp8 DoubleRow, K=256/mm)
                # batch 0: ks-outer over jt pairs to start before all chunks
                # land; later batches jt-outer so g8 casts trail each jt tile
                def mm1_drain(jt, g, g8=None, hp=None, ha=None):
                    # PSUM -> SBUF fp8 copy (mm2 lhsT); h' ~= G^2 (bf16)
                    nc.vector.tensor_copy(g8[:, jt, :], g[:])
                    if jt == 0:
                        nc.scalar.activation(hp[jt][:], g[:], AF.Square)
                    else:
                        nc.vector.scalar_tensor_tensor(
                            hp[jt][:], g[:], 1.0, g8[:, jt, :],
                            op0=ALU.mult, op1=ALU.mult,
                        )
                    if jt == 1:
                        # partial pair-sum early (slow engine, hidden)
                        nc.gpsimd.tensor_tensor(
                            ha[:], hp[0][:], hp[1][:], ALU.add
                        )

                if b == 0:
                    gtiles = {}
                    for jtg in range(2):
                        for jt in (2 * jtg, 2 * jtg + 1):
                            gt = gps.tile([128, S], F32, tag="g")
                            gtiles[jt] = gt
                        for ks in range(0, KT, 2):
                            kc, ko = divmod(ks, w)
                            for jt in (2 * jtg, 2 * jtg + 1):
                                nc.tensor.matmul(
                                    gtiles[jt][:],
                                    lhsT=qt_c[kc][:, ko:ko + 2, jt * 128:(jt + 1) * 128],
                                    rhs=pt_c[kc][:, ko:ko + 2, :],
                                    start=(ks == 0), stop=(ks == KT - 2),
                                    perf_mode=DR,
                                )
                        for jt in (2 * jtg, 2 * jtg + 1):
                            mm1_drain(jt, gtiles[jt], g8=g8, hp=hp, ha=ha)
                else:
                    for jt in range(ST):
                        g = gps.tile([128, S], F32, tag="g")
                        for ks in range(0, KT, 2):
                            nc.tensor.matmul(
                                g[:],
                                lhsT=qt_c[0][:, ks:ks + 2, jt * 128:(jt + 1) * 128],
                                rhs=pt_c[0][:, ks:ks + 2, :],
                                start=(ks == 0), stop=(ks == KT - 2),
                                perf_mode=DR,
                            )
                        mm1_drain(jt, g, g8=g8, hp=hp, ha=ha)
                        if state is not None:
                            mm2_step(state, jt)

                if state is not None:
                    dot_finish(state)
                state = {"b": b, "g8": g8, "qh": qh_t, "hp": hp, "ha": ha}

            # drain the pipeline: last batch's mm2 + dot + finals
            ha2 = scr.tile([128, S], BF16, tag="ha2")
            nc.vector.tensor_tensor(
                ha2[:], state["hp"][2][:], state["hp"][3][:], ALU.add
            )
            hs = scr.tile([128, S], BF16, tag="hs")
            nc.vector.tensor_tensor(hs[:], state["ha"][:], ha2[:], ALU.add)
            state["hs"] = hs
            for ib in range(ST):
                mm2_step(state, ib)
            dot_finish(state)
    if do_compile:
        nc.compile()
    return nc


def _get_nc():
    global _NC
    if _NC is None:
        _NC = _build()
    return _NC


F8NP = ml_dtypes.float8_e4m3


def _prep_inputs(p, q):
    p = np.asarray(p, dtype=np.float32)
    q = np.asarray(q, dtype=np.float32)

    nq = np.sqrt((q * q).sum(-1))                 # [B,S]
    srq = (1.0 / np.sqrt(nq))[..., None]          # [B,S,1]
    qs = (q * srq).astype(np.float32)
    npn = np.sqrt((p * p).sum(-1))                # [B,S]
    rpc = (1.0 / (float(D) * npn)).astype(np.float32)

    # transposed: [core, b, part, k, s] with d = k*128 + part
    def tr(x):
        x8 = x.astype(F8NP)
        return np.ascontiguousarray(
            x8.reshape(NCORES, BP, S, KT, 128).transpose(0, 1, 4, 3, 2)
        )

    # natural: [core, b, part, js, d] with j = js*128 + part
    def nat(x):
        x8 = x.astype(F8NP)
        return np.ascontiguousarray(
            x8.reshape(NCORES, BP, ST, 128, D).transpose(0, 1, 3, 2, 4)
        )

    pt8, qt8, qh8 = tr(p), tr(qs), nat(qs)
    # rpc: [core, part, b*ST + t], i = t*128 + part
    rpc_l = np.ascontiguousarray(
        rpc.reshape(NCORES, BP, ST, 128).transpose(0, 3, 1, 2)
    ).reshape(NCORES, 128, BP * ST)
    return [
        {"pt8": pt8[c], "qt8": qt8[c], "qh8": qh8[c], "rpc": rpc_l[c]}
        for c in range(NCORES)
    ]


def _postprocess(results):
    o = np.stack([np.asarray(r["out"], dtype=np.float32) for r in results])
    # o[c, part, b*ST + t] is out for batch c*BP+b at i = t*128 + part
    o = o.reshape(NCORES, 128, BP, ST).transpose(0, 2, 3, 1).reshape(B, 1, S)
    return np.ascontiguousarray(o)


def _run(inputs, trace=False, **kw):
    nc = _get_nc()
    in_maps = _prep_inputs(inputs["p"], inputs["q"])
    res = run_bass_kernel_spmd(nc, in_maps, list(range(NCORES)), trace=trace, **kw)
    return _postprocess(res.results), res


def kernel(p, q):
    out, _ = _run({"p": p, "q": q})
    return out


# revision 38
# speedup vs baseline: 1.0211x; 1.0211x over previous
"""AttentiveMatch kernel for Trainium2 (8 NeuronCores, data-parallel over batch).

Reference math (per batch):
    pn = l2norm(p); qn = l2norm(q)
    w  = -(pn @ qn^T) / D          # [S,S]
    mv = (w @ q) / S               # [S,D]
    mn = l2norm(mv)
    out = -mean(pn * mn, -1)       # [S]

Folded device pipeline (scalars folded, sign flips cancel):
    qs   = sqrt(1/|q_j|) * q_j                      (host)
    G'   = qs @ p^T                 [S,S]  fp8 DoubleRow matmul (G'[j,i])
    g8   = fp8(G')                  PSUM->SBUF copy
    dot_i = sum_j g8[j,i]^2         square (ACT) + adds + ones-matmul
    M    = sum_j g8[j,i] qs[j,d]    [i,d]  fp8 DoubleRow matmul
    ss_i = sum_d M[i,d]^2           square-accumulate along free dim
    out_i = (1/(D |p_i|)) dot_i / sqrt(ss_i)

Each core handles 8 batches; inputs shipped as fp8(e4m3) in transposed
(d-major) and natural (j-major) layouts; all accumulation fp32.
"""

import os
import sys

for _p in ("/opt/trn_rl_repo",):
    if _p not in sys.path:
        sys.path.append(_p)

import numpy as np
import ml_dtypes

import concourse.bacc as bacc
import concourse.mybir as mybir
import concourse.tile as tile
from concourse.bass_utils import run_bass_kernel_spmd

B, S, D = 64, 512, 768
NCORES = 8
BP = B // NCORES          # batches per core
ST = S // 128             # s tiles (4)
KT = D // 128             # d tiles (6)
DC = 2                    # d chunks for mm2 output (2 x 384)
DW = D // DC              # 384
F32 = mybir.dt.float32
BF16 = mybir.dt.bfloat16
F8 = mybir.dt.float8e4
AF = mybir.ActivationFunctionType
ALU = mybir.AluOpType
DR = mybir.MatmulPerfMode.DoubleRow

_NC = None


def _build(ncores=NCORES, do_compile=True):
    nc = bacc.Bacc("TRN2", target_bir_lowering=False, debug=False, num_devices=ncores)
    # transposed layouts: [b, part, k, s] with d = k*128 + part
    pt_d = nc.dram_tensor("pt8", [BP, 128, KT, S], F8, kind="ExternalInput")
    qt_d = nc.dram_tensor("qt8", [BP, 128, KT, S], F8, kind="ExternalInput")
    # natural layout: [b, part, js, d] with j = js*128 + part
    qh_d = nc.dram_tensor("qh8", [BP, 128, ST, D], F8, kind="ExternalInput")
    # 1/(D*|p_i|) at [part, b*ST + t], i = t*128 + part
    rpc_d = nc.dram_tensor("rpc", [128, BP * ST], F32, kind="ExternalInput")
    out_d = nc.dram_tensor("out", [128, BP * ST], F32, kind="ExternalOutput")

    with tile.TileContext(nc) as tc:
        with (
            tc.tile_pool(name="cst", bufs=1) as cst,
            tc.tile_pool(name="inp", bufs=3) as inp,
            tc.tile_pool(name="g8p", bufs=2) as g8p,
            tc.tile_pool(name="hpp", bufs=2) as hpp,
            tc.tile_pool(name="scr", bufs=2) as scr,
            tc.tile_pool(name="res", bufs=1) as res,
            tc.tile_pool(name="gps", bufs=3, space="PSUM") as gps,
            tc.tile_pool(name="mps", bufs=2, space="PSUM") as mps,
            tc.tile_pool(name="dps", bufs=1, space="PSUM") as dps,
        ):
            ones16 = cst.tile([128, 1], BF16)
            nc.gpsimd.memset(ones16[:], 1.0)
            rpc = cst.tile([128, BP * ST], F32)
            nc.gpsimd.dma_start(rpc[:], rpc_d[:])
            onef = cst.tile([128, 1], F32)
            nc.gpsimd.memset(onef[:], 1.0)
            warm = cst.tile([128, 1], F32)
            # PE warmup: dummy matmuls during the initial DMA wait fill the
            # idle window and trip the HAM clock-gate to full speed early
            wsrc = cst.tile([128, S], BF16)
            nc.vector.memset(wsrc[:], 1.0)
            wps = mps.tile([128, DC, 512], F32, tag="mc")
            NWARM = 7
            for i in range(NWARM):
                nc.tensor.matmul(
                    wps[0:1, 0, 0:S], lhsT=ones16[:], rhs=wsrc[:],
                    start=(i == 0), stop=(i == NWARM - 1),
                )

            # per-i dot products, transposed: col b*ST + t, i = t*128 + part
            dotT = dps.tile([128, 512], F32)
            # ss accumulator columns [i-part, b*ST + ib]
            ssc = res.tile([128, BP * ST], F32)

            # mm2 for batch st["b"], one i-block; interleaved with the NEXT
            # batch's mm1 so PSUM drains (ACT ss) get the whole iteration
            def mm2_step(st, ib):
                g8_, qh_ = st["g8"], st["qh"]
                mc = mps.tile([128, DC, 512], F32, tag="mc")
                for js in range(0, ST, 2):
                    for dc in range(DC):
                        nc.tensor.matmul(
                            mc[:, dc, 0:DW],
                            lhsT=g8_[:, js:js + 2, ib * 128:(ib + 1) * 128],
                            rhs=qh_[:, js:js + 2, dc * DW:(dc + 1) * DW],
                            start=(js == 0), stop=(js == ST - 2),
                            perf_mode=DR,
                        )
                col = st["b"] * ST + ib
                # ss: one ACT square + accumulate over both chunks (3D AP)
                s2 = scr.tile([128, DC * DW], BF16, tag="s2")
                nc.scalar.activation(
                    s2[:], mc[:, :, 0:DW], AF.Square,
                    accum_out=ssc[:, col:col + 1],
                )

            def dot_finish(st):
                bp_, hs = st["b"], st["hs"]
                for t in range(ST):
                    col = bp_ * ST + t
                    nc.tensor.matmul(
                        dotT[:, col:col + 1],
                        lhsT=hs[:, t * 128:(t + 1) * 128],
                        rhs=ones16[:],
                        start=(bp_ == 0 and t == 0),
                        stop=(bp_ == BP - 1 and t == ST - 1),
                        skip_group_check=True,
                    )
                # finals in two halves: out = dot * rpc / sqrt(ss)
                if bp_ in (BP // 2 - 1, BP - 1):
                    h0 = (0 if bp_ < BP // 2 else BP // 2) * ST
                    h1 = (bp_ + 1) * ST
                    sd = res.tile([128, BP * ST], F32, tag="sd")
                    nc.scalar.activation(sd[:, h0:h1], ssc[:, h0:h1], AF.Sqrt)
                    rs = res.tile([128, BP * ST], F32, tag="rs")
                    nc.vector.reciprocal(rs[:, h0:h1], sd[:, h0:h1])
                    w1 = res.tile([128, BP * ST], F32, tag="w1")
                    nc.vector.tensor_tensor(
                        w1[:, h0:h1], dotT[:, h0:h1], rs[:, h0:h1], ALU.mult
                    )
                    wd = res.tile([128, BP * ST], F32, tag="wd")
                    nc.vector.tensor_tensor(
                        wd[:, h0:h1], w1[:, h0:h1], rpc[:, h0:h1], ALU.mult
                    )
                    nc.sync.dma_start(out_d[:, h0:h1], wd[:, h0:h1])

            state = None
            for b in range(BP):
                # two parallel whole-tile DMAs for b0 (per-DMA completion
                # latency is ~2us and does not pipeline within a ring)
                if b == 0:
                    ring_q, ring_p = nc.sync, nc.scalar
                else:
                    ring_q = ring_p = nc.sync
                q_t = inp.tile([128, KT, S], F8, tag="qt")
                ring_q.dma_start(q_t[:], qt_d[b])
                p_t = inp.tile([128, KT, S], F8, tag="pt")
                ring_p.dma_start(p_t[:], pt_d[b])
                qh_t = inp.tile([128, ST, D], F8, tag="qh")
                # b0's qh rides the scalar ring FIFO behind the pt chunks so
                # it does not steal HBM bandwidth from the critical chunks
                (nc.scalar if b == 0 else nc.gpsimd).dma_start(qh_t[:], qh_d[b])
                if state is not None:
                    # finish prev batch's h' pair-sums on the idle engine;
                    # inputs were ready last iteration so this hides fully
                    ha2 = scr.tile([128, S], BF16, tag="ha2")
                    nc.gpsimd.tensor_tensor(
                        ha2[:], state["hp"][2][:], state["hp"][3][:], ALU.add
                    )
                    hs = scr.tile([128, S], BF16, tag="hs")
                    nc.gpsimd.tensor_tensor(hs[:], state["ha"][:], ha2[:], ALU.add)
                    state["hs"] = hs
                if b == 0:
                    # preload ACT function tables during the initial DMA wait
                    nc.scalar.activation(warm[:], onef[:], AF.Square)
                    nc.scalar.activation(warm[:], onef[:], AF.Sqrt)

                g8 = g8p.tile([128, ST, S], F8, tag="g8")
                hp0 = hpp.tile([128, S], BF16, tag="hp0")
                hp1 = hpp.tile([128, S], BF16, tag="hp1")
                hp2 = hpp.tile([128, S], BF16, tag="hp2")
                hp3 = hpp.tile([128, S], BF16, tag="hp3")
                hp = [hp0, hp1, hp2, hp3]
                ha = scr.tile([128, S], BF16, tag="ha")

                # mm1: G'[j,i] = sum_d qs[j,d] p[i,d]  (fp8 DoubleRow, K=256/mm)
                # batch 0: ks-outer over jt pairs to start before all chunks
                # land; later batches jt-outer so g8 casts trail each jt tile
                def mm1_drain(jt, g, g8=None, hp=None, ha=None):
                    # PSUM -> SBUF fp8 copy (mm2 lhsT); h' ~= G^2 (bf16)
                    nc.vector.tensor_copy(g8[:, jt, :], g[:])
                    if jt == 0:
                        nc.scalar.activation(hp[jt][:], g[:], AF.Square)
                    else:
                        nc.vector.scalar_tensor_tensor(
                            hp[jt][:], g[:], 1.0, g8[:, jt, :],
                            op0=ALU.mult, op1=ALU.mult,
                        )
                    if jt == 1:
                        # partial pair-sum early (slow engine, hidden)
                        nc.gpsimd.tensor_tensor(
                            ha[:], hp[0][:], hp[1][:], ALU.add
                        )

                for jt in range(ST):
                    g = gps.tile([128, S], F32, tag="g")
                    for ks in range(0, KT, 2):
                        nc.tensor.matmul(
                            g[:],
                            lhsT=q_t[:, ks:ks + 2, jt * 128:(jt + 1) * 128],
                            rhs=p_t[:, ks:ks + 2, :],
                            start=(ks == 0), stop=(ks == KT - 2),
                            perf_mode=DR,
                        )
                    mm1_drain(jt, g, g8=g8, hp=hp, ha=ha)
                    if state is not None:
                        mm2_step(state, jt)

                if state is not None:
                    dot_finish(state)
                state = {"b": b, "g8": g8, "qh": qh_t, "hp": hp, "ha": ha}

            # drain the pipeline: last batch's mm2 + dot + finals
            ha2 = scr.tile([128, S], BF16, tag="ha2")
            nc.vector.tensor_tensor(
                ha2[:], state["hp"][2][:], state["hp"][3][:], ALU.add
            )
            hs = scr.tile([128, S], BF16, tag="hs")
            nc.vector.tensor_tensor(hs[:], state["ha"][:], ha2[:], ALU.add)
            state["hs"] = hs
            for ib in range(ST):
                mm2_step(state, ib)
            dot_finish(state)
    if do_compile:
        nc.compile()
    return nc


def _get_nc():
    global _NC
    if _NC is None:
        _NC = _build()
    return _NC


F8NP = ml_dtypes.float8_e4m3


def _prep_inputs(p, q):
    p = np.asarray(p, dtype=np.float32)
    q = np.asarray(q, dtype=np.float32)

    nq = np.sqrt((q * q).sum(-1))                 # [B,S]
    srq = (1.0 / np.sqrt(nq))[..., None]          # [B,S,1]
    qs = (q * srq).astype(np.float32)
    npn = np.sqrt((p * p).sum(-1))                # [B,S]
    rpc = (1.0 / (float(D) * npn)).astype(np.float32)

    # transposed: [core, b, part, k, s] with d = k*128 + part
    def tr(x):
        x8 = x.astype(F8NP)
        return np.ascontiguousarray(
            x8.reshape(NCORES, BP, S, KT, 128).transpose(0, 1, 4, 3, 2)
        )

    # natural: [core, b, part, js, d] with j = js*128 + part
    def nat(x):
        x8 = x.astype(F8NP)
        return np.ascontiguousarray(
            x8.reshape(NCORES, BP, ST, 128, D).transpose(0, 1, 3, 2, 4)
        )

    pt8, qt8, qh8 = tr(p), tr(qs), nat(qs)
    # rpc: [core, part, b*ST + t], i = t*128 + part
    rpc_l = np.ascontiguousarray(
        rpc.reshape(NCORES, BP, ST, 128).transpose(0, 3, 1, 2)
    ).reshape(NCORES, 128, BP * ST)
    return [
        {"pt8": pt8[c], "qt8": qt8[c], "qh8": qh8[c], "rpc": rpc_l[c]}
        for c in range(NCORES)
    ]


def _postprocess(results):
    o = np.stack([np.asarray(r["out"], dtype=np.float32) for r in results])
    # o[c, part, b*ST + t] is out for batch c*BP+b at i = t*128 + part
    o = o.reshape(NCORES, 128, BP, ST).transpose(0, 2, 3, 1).reshape(B, 1, S)
    return np.ascontiguousarray(o)


def _run(inputs, trace=False, **kw):
    nc = _get_nc()
    in_maps = _prep_inputs(inputs["p"], inputs["q"])
    res = run_bass_kernel_spmd(nc, in_maps, list(range(NCORES)), trace=trace, **kw)
    return _postprocess(res.results), res


def kernel(p, q):
    out, _ = _run({"p": p, "q": q})
    return out
